# revision 23
# baseline (speedup 1.0000x reference)
"""Causal self-attention on 8 Trainium2 NeuronCores.

Sharding (batch + head parallel): core c handles batch b = c // 4 and the
4 heads [hg*4, hg*4+4) where hg = c % 4.  Each core computes q/k/v from
column-sliced c_attn weights, full causal attention for its heads, and a
partial c_proj output from the matching row slice of w_proj; the host
sums the 4 partials per batch.

v2 vs baseline: all matmul operands in bf16 (PSUM stays fp32), the
S->exp->PV chain is software-pipelined at (head, k-pair) unit granularity
with S issued 3 units ahead of PV, qkv for block b+1 and c_proj for block
b-1 are interleaved as PE filler inside attention block b, the softmax
denominator broadcast uses gpsimd partition_broadcast instead of a PE
matmul, input DMA is chunked so the first matmul starts early, and the
partial output is returned as bf16 (summed in fp64 on host).
"""

import sys

if "/opt/trn_rl_repo" not in sys.path:
    sys.path.insert(0, "/opt/trn_rl_repo")

import ml_dtypes
import numpy as np

import concourse.mybir as mybir
from concourse import bacc
from concourse.bass_utils import run_bass_kernel_spmd
from concourse.tile import TileContext

B, T, C = 2, 2048, 1024
H, D = 16, 64
HL = 4  # heads per core
N_CORES = 8
KT = C // 128  # contraction tiles over the embedding dim
SCALE = 1.0 / 8.0  # 1/sqrt(D)
LOOKAHEAD = 4  # S units issued ahead of PV consumption

_CACHE = {}


def _build():
    f32 = mybir.dt.float32
    bf16 = mybir.dt.bfloat16
    nc = bacc.Bacc("TRN2", target_bir_lowering=False, debug=False, num_devices=N_CORES)

    x_in = nc.dram_tensor("x_in", [128, KT, T], bf16, kind="ExternalInput")
    wqk = nc.dram_tensor("wqk", [128, KT, 2 * HL * D], bf16, kind="ExternalInput")
    wv = nc.dram_tensor("wv", [128, KT, HL * D], bf16, kind="ExternalInput")
    wp = nc.dram_tensor("wp", [128, HL // 2, C], bf16, kind="ExternalInput")
    out = nc.dram_tensor("out", [T, C], bf16, kind="ExternalOutput")

    with TileContext(nc) as tc:
        with tc.tile_pool(name="persist", bufs=1) as persist:
            # q/k feature-major [d, t]: slot 0/1 = q heads {0,1}/{2,3}, 2/3 = k;
            # one tile per 512-token block for fine-grained cross-stage deps
            qk_t = [
                [persist.tile([128, 512], bf16, name=f"qk{s}_{tb}") for tb in range(4)]
                for s in range(4)
            ]
            # v token-major per 128-token tile; col D holds ones (denominator)
            v_t = [
                persist.tile([128, HL, D + 1], bf16, name=f"v{tt}") for tt in range(16)
            ]
            # head-pair stacked normalized y per 512-token block
            y2_t = [
                persist.tile([128, HL // 2, 512], bf16, name=f"y2{b_}")
                for b_ in range(4)
            ]
            wp_sb = persist.tile([128, HL // 2, C], bf16)

            # startup DMA: interleave x block-0 and wqk chunks so the kt=0
            # pieces (which gate the first matmul) land first
            tri32 = persist.tile([128, 128], f32)
            tri = persist.tile([128, 128], bf16)
            # ones row for the K=1 denominator-broadcast matmul
            onesbc = persist.tile([1, D], bf16)
            # [0 | tri] mask for the fused diagonal-pair exp (s=1 slice)
            mask256 = persist.tile([128, 256], bf16)

            with (
                tc.tile_pool(name="qkvp", bufs=1) as qkvp,
                tc.tile_pool(name="attp", bufs=6) as attp,
                tc.tile_pool(name="attsmall", bufs=2) as attsmall,
                tc.tile_pool(name="projp", bufs=3) as projp,
                tc.tile_pool(name="ps_st", bufs=2, space="PSUM") as ps_st,
                tc.tile_pool(name="ps_y", bufs=2, space="PSUM") as ps_y,
                tc.tile_pool(name="ps_share", bufs=2, space="PSUM") as ps_share,
            ):
                # quarter-length x buffers, double-buffered across stages
                x_q = [
                    qkvp.tile([128, KT, T // 4], bf16, name=f"x_q{i}")
                    for i in range(2)
                ]
                wqk_sb = qkvp.tile([128, KT, 2 * HL * D], bf16)
                wv_sb = qkvp.tile([128, KT, HL * D], bf16)

                for kt in range(KT):
                    nc.sync.dma_start(x_q[0][:, kt, :], x_in[:, kt, 0:512])
                    nc.sync.dma_start(wqk_sb[:, kt, :], wqk[:, kt, :])
                for kt in range(KT):
                    nc.sync.dma_start(wv_sb[:, kt, :], wv[:, kt, :])
                for i in range(4):
                    nc.sync.dma_start(
                        wp_sb[:, :, i * 256 : (i + 1) * 256],
                        wp[:, :, i * 256 : (i + 1) * 256],
                    )

                # lower-triangular 0/1 mask for the diagonal 128x128 blocks
                nc.vector.memset(tri32, 1.0)
                nc.gpsimd.affine_select(
                    out=tri32,
                    in_=tri32,
                    pattern=[[1, 128]],
                    channel_multiplier=-1,
                    base=0,
                    compare_op=mybir.AluOpType.is_ge,
                    fill=0.0,
                )
                nc.vector.tensor_copy(tri, tri32)
                nc.vector.memset(onesbc, 1.0)
                nc.vector.memset(mask256[:, 0:128], 0.0)
                nc.vector.tensor_copy(mask256[:, 128:256], tri32)
                for tt in range(16):
                    nc.vector.memset(v_t[tt][:, :, D : D + 1], 1.0)

                def qkv_dma(tb):
                    x_sb = x_q[tb % 2]
                    for kt in range(KT):
                        nc.sync.dma_start(
                            x_sb[:, kt, :], x_in[:, kt, tb * 512 : (tb + 1) * 512]
                        )

                def qk_group(tb, jt, on_act=False):
                    x_sb = x_q[tb % 2]
                    qk_ps = ps_share.tile([128, 512], f32, tag="share", name="qk_ps")
                    for kt in range(KT):
                        nc.tensor.matmul(
                            qk_ps,
                            wqk_sb[:, kt, jt * 128 : (jt + 1) * 128],
                            x_sb[:, kt, :],
                            start=(kt == 0),
                            stop=(kt == KT - 1),
                        )
                    if on_act:
                        nc.scalar.copy(qk_t[jt][tb], qk_ps)
                    else:
                        nc.vector.tensor_copy(qk_t[jt][tb], qk_ps)

                def v_group(tb, tt2, on_act=False):
                    x_sb = x_q[tb % 2]
                    tt = tb * 4 + tt2
                    v_ps = ps_share.tile([128, 512], f32, tag="share", name="v_ps")
                    for kt in range(KT):
                        nc.tensor.matmul(
                            v_ps[:, 0 : HL * D],
                            x_sb[:, kt, tt2 * 128 : (tt2 + 1) * 128],
                            wv_sb[:, kt, :],
                            start=(kt == 0),
                            stop=(kt == KT - 1),
                        )
                    src = v_ps[:, 0 : HL * D].rearrange("p (h d) -> p h d", h=HL)
                    if on_act:
                        nc.scalar.copy(v_t[tt][:, :, 0:D], src)
                    else:
                        nc.vector.tensor_copy(v_t[tt][:, :, 0:D], src)

                def proj_group(tt, cb, on_act=False, drain=False):
                    # c_proj half-row for token tile tt, column block cb
                    blk, off = tt // 4, (tt % 4) * 128
                    if drain and (tt * 2 + cb) % 2 == 1:
                        # attention PSUM is idle in the drain: borrow an st
                        # tile so four proj groups pipeline instead of two
                        o_ps = ps_st.tile([128, 2, 512], f32, name="st")[:, 0, :]
                    else:
                        o_ps = ps_share.tile([128, 512], f32, tag="share", name="o_ps")
                    for pr in range(HL // 2):
                        nc.tensor.matmul(
                            o_ps,
                            y2_t[blk][:, pr, off : off + 128],
                            wp_sb[:, pr, cb * 512 : (cb + 1) * 512],
                            start=(pr == 0),
                            stop=(pr == HL // 2 - 1),
                        )
                    o_sb = projp.tile([128, 512], bf16, tag="o", name="o_sb")
                    if on_act:
                        nc.scalar.copy(o_sb, o_ps)
                    else:
                        nc.vector.tensor_copy(o_sb, o_ps)
                    nsplit = 4 if drain else 2
                    wq = 512 // nsplit
                    for q in range(nsplit):
                        nc.sync.dma_start(
                            out[
                                tt * 128 : (tt + 1) * 128,
                                cb * 512 + q * wq : cb * 512 + (q + 1) * wq,
                            ],
                            o_sb[:, q * wq : (q + 1) * wq],
                        )

                def s_unit(jq, h, p):
                    qslot, kslot = h // 2, 2 + h // 2
                    base = (h % 2) * D
                    st = ps_st.tile([128, 2, 512], f32, name="st")
                    est = attp.tile([128, 2, 512], bf16, tag="est", name="est")
                    diag = 2 * p >= 4 * jq
                    # on diagonal pairs both S matmuls start at w0 so one
                    # rectangular exp covers the pair; the s=1 columns
                    # [w0, w0+128) are valid above-diagonal scores that
                    # mask256 then zeroes
                    w0 = max(0, (2 * p - 4 * jq) * 128)
                    for s in range(2):
                        j = 2 * p + s
                        nc.tensor.matmul(
                            st[:, s, w0:],
                            qk_t[kslot][j // 4][
                                base : base + D,
                                (j % 4) * 128 : (j % 4 + 1) * 128,
                            ],
                            qk_t[qslot][jq][base : base + D, w0:],
                            start=True,
                            stop=True,
                        )
                    nc.scalar.activation(
                        est[:, :, w0:],
                        st[:, :, w0:],
                        mybir.ActivationFunctionType.Exp,
                        scale=SCALE,
                    )
                    if diag:
                        # masking on the otherwise-idle gpsimd engine keeps
                        # the DVE queue out of the PV critical path
                        nc.gpsimd.tensor_mul(
                            est[:, 0, w0 : w0 + 128],
                            est[:, 0, w0 : w0 + 128],
                            tri,
                        )
                        nc.gpsimd.tensor_mul(
                            est[:, 1, w0 : w0 + 256],
                            est[:, 1, w0 : w0 + 256],
                            mask256,
                        )
                    return est

                def pv_unit(jq, h, p, est, y_ps):
                    njt = 4 * (jq + 1)
                    for s in range(2):
                        j = 2 * p + s
                        w = max(0, (j - 4 * jq) * 128)
                        nc.tensor.matmul(
                            y_ps[:, w:],
                            v_t[j][:, h, :],
                            est[:, s, w:],
                            start=(j == 0),
                            stop=(j == njt - 1),
                        )

                def epilogue_a(jq, h, y_ps):
                    # copy the denominator row off PSUM (DVE) so the later
                    # broadcast matmul doesn't stall the PE on this copy
                    r = attsmall.tile([1, 512], bf16, tag="r", name="r")
                    nc.vector.tensor_copy(r, y_ps[D : D + 1, :])
                    return r

                def epilogue_b(jq, h, y_ps, r):
                    pr = h // 2
                    rb_ps = ps_share.tile([128, 512], f32, tag="share", name="rb_ps")
                    nc.tensor.matmul(rb_ps[0:D, :], onesbc, r, start=True, stop=True)
                    rb = attsmall.tile([D, 512], f32, tag="rb", name="rb")
                    nc.vector.reciprocal_approx_fast(rb, rb_ps[0:D, :])
                    if h % 2 == 0:
                        nc.vector.tensor_mul(
                            y2_t[jq][0:D, pr, :], y_ps[0:D, :], rb
                        )
                    else:
                        ylo = attsmall.tile([D, 512], bf16, tag="ylo", name="ylo")
                        nc.vector.tensor_mul(ylo, y_ps[0:D, :], rb)
                        nc.gpsimd.dma_start(y2_t[jq][D:128, pr, :], ylo)

                # stage 0 qkv up front; q/k copies on the (otherwise idle)
                # ACT, v copies on DVE (v is consumed later, by PV)
                for jt in range(4):
                    qk_group(0, jt, on_act=True)
                for tt2 in range(4):
                    v_group(0, tt2)

                # odd heads first so their partition-shift DMA is off the
                # critical path into c_proj; last head's epilogue is a plain
                # DVE mul
                HEAD_ORDER = [1, 3, 0, 2]

                for jq in range(4):
                    npair = 2 * (jq + 1)
                    units = [(h, p) for h in HEAD_ORDER for p in range(npair)]
                    if jq < 3:
                        qkv_dma(jq + 1)
                    # v of the current block leads (its PVs arrive within a
                    # few units); q/k of the next block must land before that
                    # block's attention starts; proj of the previous block
                    # has a full block of slack
                    fillers = [
                        lambda t2=t2: v_group(jq, t2) for t2 in range(4)
                    ] if jq >= 1 else []
                    qk_fill = (
                        [lambda jt=jt: qk_group(jq + 1, jt) for jt in range(4)]
                        if jq < 3
                        else []
                    )
                    # c_proj groups: blocks 0 and 2 split 8 into their next
                    # attention block; block 1 splits 4/4 between attn(2) and
                    # attn(3) so the (filler-poor, ACT-paced) last block stays
                    # fed
                    proj_items = []
                    if jq == 1:
                        proj_items = [(tt, cb) for tt in range(0, 4) for cb in range(2)]
                    elif jq == 2:
                        proj_items = [(tt, cb) for tt in range(4, 8) for cb in range(2)][:4]
                    elif jq == 3:
                        proj_items = (
                            [(tt, cb) for tt in range(4, 8) for cb in range(2)][4:]
                            + [(tt, cb) for tt in range(8, 12) for cb in range(2)]
                        )
                    proj_fill = [
                        lambda tt=tt, cb=cb: proj_group(tt, cb)
                        for tt, cb in proj_items
                    ]
                    while qk_fill or proj_fill:
                        if qk_fill:
                            fillers.append(qk_fill.pop(0))
                        if proj_fill:
                            fillers.append(proj_fill.pop(0))

                    slots = len(units) + LOOKAHEAD
                    # last block: bias fillers toward the tail, where the
                    # exp-paced stream otherwise leaves the PE short of work
                    if jq == 3 and fillers:
                        nf = len(fillers)
                        # v fillers early (diag PVs need them), proj late
                        due = [
                            2 * k if k < 4 else max(k, slots - 2 * (nf - k))
                            for k in range(nf)
                        ]
                    else:
                        due = [
                            (k * slots) // max(len(fillers), 1)
                            for k in range(len(fillers))
                        ]
                    est_q = {}
                    y_by_head = {}
                    pending = []
                    fi = 0
                    for i in range(slots):
                        if i < len(units):
                            h, p = units[i]
                            est_q[i] = s_unit(jq, h, p)
                        if i >= LOOKAHEAD:
                            h, p = units[i - LOOKAHEAD]
                            if p == 0:
                                y_by_head[h] = ps_y.tile(
                                    [D + 1, 512], f32, name="y_ps"
                                )
                            pv_unit(jq, h, p, est_q.pop(i - LOOKAHEAD), y_by_head[h])
                            if p == npair - 1:
                                y_ps = y_by_head.pop(h)
                                r = epilogue_a(jq, h, y_ps)
                                pending.append(
                                    [i + 2, lambda jq=jq, h=h, y_ps=y_ps, r=r:
                                        epilogue_b(jq, h, y_ps, r)]
                                )
                        while pending and pending[0][0] <= i:
                            pending.pop(0)[1]()
                        while fi < len(fillers) and due[fi] <= i:
                            fillers[fi]()
                            fi += 1
                    for _, fn in pending:
                        fn()

                # drain: c_proj for the last token block; copies alternate
                # between ACT (exp work done) and DVE, four PSUM tiles in
                # flight
                for k, (tt, cb) in enumerate(
                    [(tt, cb) for tt in range(12, 16) for cb in range(2)]
                ):
                    proj_group(tt, cb, on_act=(k % 2 == 0), drain=True)

    nc.compile()
    return nc


def _get_nc():
    if "nc" not in _CACHE:
        _CACHE["nc"] = _build()
    return _CACHE["nc"]


def make_in_maps(x, w_attn, w_proj):
    bf = ml_dtypes.bfloat16
    x = np.asarray(x, np.float32)
    w_attn = np.asarray(w_attn, np.float32)
    w_proj = np.asarray(w_proj, np.float32)
    in_maps = []
    for c in range(N_CORES):
        b, hg = c // 4, c % 4
        hs = hg * HL * D  # 256 * hg
        xt = np.ascontiguousarray(x[b].T)  # [C, T]
        x_t = xt.reshape(KT, 128, T).transpose(1, 0, 2)
        wq = w_attn[hs : hs + HL * D, :]
        wk = w_attn[C + hs : C + hs + HL * D, :]
        wqkt = np.concatenate([wq, wk], 0).T  # [C, 512]
        wqk_t = wqkt.reshape(KT, 128, 2 * HL * D).transpose(1, 0, 2)
        wvt = w_attn[2 * C + hs : 2 * C + hs + HL * D, :].T  # [C, 256]
        wv_t = wvt.reshape(KT, 128, HL * D).transpose(1, 0, 2)
        # head-pair stacked rows: [128, HL//2, C]; partition p of pair pr is
        # local feature pr*128 + p (head 2*pr dims then head 2*pr+1 dims)
        wp_t = (
            w_proj[:, hs : hs + HL * D].T.reshape(HL // 2, 128, C).transpose(1, 0, 2)
        )
        in_maps.append(
            {
                "x_in": np.ascontiguousarray(x_t).astype(bf),
                "wqk": np.ascontiguousarray(wqk_t).astype(bf),
                "wv": np.ascontiguousarray(wv_t).astype(bf),
                "wp": np.ascontiguousarray(wp_t).astype(bf),
            }
        )
    return in_maps


def run(in_maps, **kwargs):
    nc = _get_nc()
    return run_bass_kernel_spmd(nc, in_maps, core_ids=list(range(N_CORES)), **kwargs)


def combine(results):
    out = np.zeros((B, T, C), np.float64)
    for c in range(N_CORES):
        out[c // 4] += results[c]["out"].astype(np.float64)
    return out.astype(np.float32)


def kernel(x, w_attn, w_proj):
    res = run(make_in_maps(x, w_attn, w_proj))
    return combine(res.results)


# revision 24
# speedup vs baseline: 1.0419x; 1.0419x over previous
"""Causal self-attention on 8 Trainium2 NeuronCores.

Sharding (batch + head parallel): core c handles batch b = c // 4 and the
4 heads [hg*4, hg*4+4) where hg = c % 4.  Each core computes q/k/v from
column-sliced c_attn weights, full causal attention for its heads, and a
partial c_proj output from the matching row slice of w_proj; the host
sums the 4 partials per batch.

All matmul operands are bf16 (PSUM accumulates fp32).  The S->exp->PV
chain is software-pipelined at (head, k-pair) unit granularity with S
issued LOOKAHEAD units ahead of PV; qkv for block b+1 and c_proj for
block b-1 interleave as PE filler inside attention block b.  Input DMA is
chunked per contraction tile so the first matmul starts early, and the
partial output is returned as bf16 (summed in fp64 on host).
"""

import sys

if "/opt/trn_rl_repo" not in sys.path:
    sys.path.insert(0, "/opt/trn_rl_repo")

import ml_dtypes
import numpy as np

import concourse.mybir as mybir
from concourse import bacc
from concourse.bass_utils import run_bass_kernel_spmd
from concourse.tile import TileContext

B, T, C = 2, 2048, 1024
H, D = 16, 64
HL = 4  # heads per core
N_CORES = 8
KT = C // 128  # contraction tiles over the embedding dim
SCALE = 1.0 / 8.0  # 1/sqrt(D)
LOOKAHEAD = 3  # S units issued ahead of PV consumption

_CACHE = {}


def _build():
    f32 = mybir.dt.float32
    bf16 = mybir.dt.bfloat16
    nc = bacc.Bacc("TRN2", target_bir_lowering=False, debug=False, num_devices=N_CORES)

    x_in = nc.dram_tensor("x_in", [128, KT, T], bf16, kind="ExternalInput")
    wqk = nc.dram_tensor("wqk", [128, KT, 2 * HL * D], bf16, kind="ExternalInput")
    wv = nc.dram_tensor("wv", [128, KT, HL * D], bf16, kind="ExternalInput")
    wp = nc.dram_tensor("wp", [128, HL // 2, C], bf16, kind="ExternalInput")
    out = nc.dram_tensor("out", [T, C], bf16, kind="ExternalOutput")

    with TileContext(nc) as tc:
        with tc.tile_pool(name="persist", bufs=1) as persist:
            # q/k feature-major [d, t]: slot 0/1 = q heads {0,1}/{2,3}, 2/3 = k;
            # one tile per 512-token block for fine-grained cross-stage deps
            qk_t = [
                [persist.tile([128, 512], bf16, name=f"qk{s}_{tb}") for tb in range(4)]
                for s in range(4)
            ]
            # v token-major per 128-token tile; col D holds ones (denominator)
            v_t = [
                persist.tile([128, HL, D + 1], bf16, name=f"v{tt}") for tt in range(16)
            ]
            # head-pair stacked normalized y per 512-token block
            y2_t = [
                persist.tile([128, HL // 2, 512], bf16, name=f"y2{b_}")
                for b_ in range(4)
            ]
            wp_sb = persist.tile([128, HL // 2, C], bf16)

            tri32 = persist.tile([128, 128], f32)
            tri = persist.tile([128, 128], bf16)
            # ones row for the K=1 denominator-broadcast matmul
            onesbc = persist.tile([1, D], bf16)

            with (
                tc.tile_pool(name="qkvp", bufs=1) as qkvp,
                tc.tile_pool(name="attp", bufs=6) as attp,
                tc.tile_pool(name="attsmall", bufs=2) as attsmall,
                tc.tile_pool(name="projp", bufs=3) as projp,
                tc.tile_pool(name="ps_st", bufs=2, space="PSUM") as ps_st,
                tc.tile_pool(name="ps_y", bufs=2, space="PSUM") as ps_y,
                tc.tile_pool(name="ps_share", bufs=2, space="PSUM") as ps_share,
            ):
                # quarter-length x buffers, double-buffered across stages
                x_q = [
                    qkvp.tile([128, KT, T // 4], bf16, name=f"x_q{i}")
                    for i in range(2)
                ]
                wqk_sb = qkvp.tile([128, KT, 2 * HL * D], bf16)
                wv_sb = qkvp.tile([128, KT, HL * D], bf16)

                # startup DMA: interleave x block-0 and wqk chunks so the
                # kt=0 pieces (which gate the first matmul) land first
                for kt in range(KT):
                    nc.sync.dma_start(x_q[0][:, kt, :], x_in[:, kt, 0:512])
                    nc.sync.dma_start(wqk_sb[:, kt, :], wqk[:, kt, :])
                for kt in range(KT):
                    nc.sync.dma_start(wv_sb[:, kt, :], wv[:, kt, :])
                for i in range(4):
                    nc.sync.dma_start(
                        wp_sb[:, :, i * 256 : (i + 1) * 256],
                        wp[:, :, i * 256 : (i + 1) * 256],
                    )

                # lower-triangular 0/1 mask for the diagonal 128x128 blocks
                nc.vector.memset(tri32, 1.0)
                nc.gpsimd.affine_select(
                    out=tri32,
                    in_=tri32,
                    pattern=[[1, 128]],
                    channel_multiplier=-1,
                    base=0,
                    compare_op=mybir.AluOpType.is_ge,
                    fill=0.0,
                )
                nc.vector.tensor_copy(tri, tri32)
                nc.vector.memset(onesbc, 1.0)
                for tt in range(16):
                    nc.vector.memset(v_t[tt][:, :, D : D + 1], 1.0)

                def qkv_dma(tb):
                    x_sb = x_q[tb % 2]
                    for kt in range(KT):
                        nc.sync.dma_start(
                            x_sb[:, kt, :], x_in[:, kt, tb * 512 : (tb + 1) * 512]
                        )

                def qk_group(tb, jt, on_act=False):
                    x_sb = x_q[tb % 2]
                    qk_ps = ps_share.tile([128, 512], f32, tag="share", name="qk_ps")
                    for kt in range(KT):
                        nc.tensor.matmul(
                            qk_ps,
                            wqk_sb[:, kt, jt * 128 : (jt + 1) * 128],
                            x_sb[:, kt, :],
                            start=(kt == 0),
                            stop=(kt == KT - 1),
                        )
                    if on_act:
                        nc.scalar.copy(qk_t[jt][tb], qk_ps)
                    else:
                        nc.vector.tensor_copy(qk_t[jt][tb], qk_ps)

                def v_group(tb, tt2, on_act=False):
                    x_sb = x_q[tb % 2]
                    tt = tb * 4 + tt2
                    v_ps = ps_share.tile([128, 512], f32, tag="share", name="v_ps")
                    for kt in range(KT):
                        nc.tensor.matmul(
                            v_ps[:, 0 : HL * D],
                            x_sb[:, kt, tt2 * 128 : (tt2 + 1) * 128],
                            wv_sb[:, kt, :],
                            start=(kt == 0),
                            stop=(kt == KT - 1),
                        )
                    src = v_ps[:, 0 : HL * D].rearrange("p (h d) -> p h d", h=HL)
                    if on_act:
                        nc.scalar.copy(v_t[tt][:, :, 0:D], src)
                    else:
                        nc.vector.tensor_copy(v_t[tt][:, :, 0:D], src)

                def proj_group(tt, cb):
                    # c_proj half-row for token tile tt, column block cb
                    blk, off = tt // 4, (tt % 4) * 128
                    o_ps = ps_share.tile([128, 512], f32, tag="share", name="o_ps")
                    for pr in range(HL // 2):
                        nc.tensor.matmul(
                            o_ps,
                            y2_t[blk][:, pr, off : off + 128],
                            wp_sb[:, pr, cb * 512 : (cb + 1) * 512],
                            start=(pr == 0),
                            stop=(pr == HL // 2 - 1),
                        )
                    o_sb = projp.tile([128, 512], bf16, tag="o", name="o_sb")
                    nc.vector.tensor_copy(o_sb, o_ps)
                    for q in range(2):
                        nc.sync.dma_start(
                            out[
                                tt * 128 : (tt + 1) * 128,
                                cb * 512 + q * 256 : cb * 512 + (q + 1) * 256,
                            ],
                            o_sb[:, q * 256 : (q + 1) * 256],
                        )

                def s_unit(jq, h, p):
                    qslot, kslot = h // 2, 2 + h // 2
                    base = (h % 2) * D
                    st = ps_st.tile([128, 2, 512], f32, name="st")
                    est = attp.tile([128, 2, 512], bf16, tag="est", name="est")
                    diag = 2 * p >= 4 * jq
                    for s in range(2):
                        j = 2 * p + s
                        w = max(0, (j - 4 * jq) * 128)
                        nc.tensor.matmul(
                            st[:, s, w:],
                            qk_t[kslot][j // 4][
                                base : base + D,
                                (j % 4) * 128 : (j % 4 + 1) * 128,
                            ],
                            qk_t[qslot][jq][base : base + D, w:],
                            start=True,
                            stop=True,
                        )
                    if not diag:
                        nc.scalar.activation(
                            est, st, mybir.ActivationFunctionType.Exp, scale=SCALE
                        )
                    else:
                        for s in range(2):
                            j = 2 * p + s
                            w = max(0, (j - 4 * jq) * 128)
                            nc.scalar.activation(
                                est[:, s, w:],
                                st[:, s, w:],
                                mybir.ActivationFunctionType.Exp,
                                scale=SCALE,
                            )
                            nc.vector.tensor_mul(
                                est[:, s, w : w + 128],
                                est[:, s, w : w + 128],
                                tri,
                            )
                    return est

                def pv_unit(jq, h, p, est, y_ps):
                    njt = 4 * (jq + 1)
                    for s in range(2):
                        j = 2 * p + s
                        w = max(0, (j - 4 * jq) * 128)
                        nc.tensor.matmul(
                            y_ps[:, w:],
                            v_t[j][:, h, :],
                            est[:, s, w:],
                            start=(j == 0),
                            stop=(j == njt - 1),
                        )

                def epilogue_a(jq, h, y_ps):
                    # copy the denominator row off PSUM (DVE) so the later
                    # broadcast matmul doesn't stall the PE on this copy
                    r = attsmall.tile([1, 512], bf16, tag="r", name="r")
                    nc.vector.tensor_copy(r, y_ps[D : D + 1, :])
                    return r

                def epilogue_b(jq, h, y_ps, r):
                    pr = h // 2
                    rb_ps = ps_share.tile([128, 512], f32, tag="share", name="rb_ps")
                    nc.tensor.matmul(rb_ps[0:D, :], onesbc, r, start=True, stop=True)
                    rb = attsmall.tile([D, 512], f32, tag="rb", name="rb")
                    nc.vector.reciprocal_approx_fast(rb, rb_ps[0:D, :])
                    if h % 2 == 0:
                        nc.vector.tensor_mul(
                            y2_t[jq][0:D, pr, :], y_ps[0:D, :], rb
                        )
                    else:
                        ylo = attsmall.tile([D, 512], bf16, tag="ylo", name="ylo")
                        nc.vector.tensor_mul(ylo, y_ps[0:D, :], rb)
                        nc.gpsimd.dma_start(y2_t[jq][D:128, pr, :], ylo)

                # stage 0 qkv up front; copies on the (otherwise idle) ACT
                for jt in range(4):
                    qk_group(0, jt, on_act=True)
                for tt2 in range(4):
                    v_group(0, tt2, on_act=True)

                for jq in range(4):
                    npair = 2 * (jq + 1)
                    units = [(h, p) for h in range(HL) for p in range(npair)]
                    if jq < 3:
                        qkv_dma(jq + 1)
                    fillers = []
                    qkv_fill = (
                        [lambda jt=jt: qk_group(jq + 1, jt) for jt in range(4)]
                        + [lambda t2=t2: v_group(jq + 1, t2) for t2 in range(4)]
                        if jq < 3
                        else []
                    )
                    proj_fill = (
                        [
                            lambda tt=tt, cb=cb: proj_group(tt, cb)
                            for tt in range(4 * (jq - 1), 4 * jq)
                            for cb in range(2)
                        ]
                        if jq >= 1
                        else []
                    )
                    # k/v of the next block must finish before its attention
                    # starts; proj of the previous block has a full block of
                    # slack — alternate, qkv first
                    while qkv_fill or proj_fill:
                        if qkv_fill:
                            fillers.append(qkv_fill.pop(0))
                        if proj_fill:
                            fillers.append(proj_fill.pop(0))

                    slots = len(units) + LOOKAHEAD
                    est_q = {}
                    y_by_head = {}
                    pending = []
                    fi = 0
                    for i in range(slots):
                        if i < len(units):
                            h, p = units[i]
                            est_q[i] = s_unit(jq, h, p)
                        if i >= LOOKAHEAD:
                            h, p = units[i - LOOKAHEAD]
                            if p == 0:
                                y_by_head[h] = ps_y.tile(
                                    [D + 1, 512], f32, name="y_ps"
                                )
                            pv_unit(jq, h, p, est_q.pop(i - LOOKAHEAD), y_by_head[h])
                            if p == npair - 1:
                                y_ps = y_by_head.pop(h)
                                r = epilogue_a(jq, h, y_ps)
                                pending.append(
                                    [i + 2, lambda jq=jq, h=h, y_ps=y_ps, r=r:
                                        epilogue_b(jq, h, y_ps, r)]
                                )
                        while pending and pending[0][0] <= i:
                            pending.pop(0)[1]()
                        while fi < len(fillers) and fi * slots <= i * len(fillers):
                            fillers[fi]()
                            fi += 1
                    for _, fn in pending:
                        fn()

                # drain: c_proj for the last token block
                for tt in range(12, 16):
                    for cb in range(2):
                        proj_group(tt, cb)

    nc.compile()
    return nc


def _get_nc():
    if "nc" not in _CACHE:
        _CACHE["nc"] = _build()
    return _CACHE["nc"]


def make_in_maps(x, w_attn, w_proj):
    bf = ml_dtypes.bfloat16
    x = np.asarray(x, np.float32)
    w_attn = np.asarray(w_attn, np.float32)
    w_proj = np.asarray(w_proj, np.float32)
    in_maps = []
    for c in range(N_CORES):
        b, hg = c // 4, c % 4
        hs = hg * HL * D  # 256 * hg
        xt = np.ascontiguousarray(x[b].T)  # [C, T]
        x_t = xt.reshape(KT, 128, T).transpose(1, 0, 2)
        wq = w_attn[hs : hs + HL * D, :]
        wk = w_attn[C + hs : C + hs + HL * D, :]
        wqkt = np.concatenate([wq, wk], 0).T  # [C, 512]
        wqk_t = wqkt.reshape(KT, 128, 2 * HL * D).transpose(1, 0, 2)
        wvt = w_attn[2 * C + hs : 2 * C + hs + HL * D, :].T  # [C, 256]
        wv_t = wvt.reshape(KT, 128, HL * D).transpose(1, 0, 2)
        # head-pair stacked rows: [128, HL//2, C]; partition p of pair pr is
        # local feature pr*128 + p (head 2*pr dims then head 2*pr+1 dims)
        wp_t = (
            w_proj[:, hs : hs + HL * D].T.reshape(HL // 2, 128, C).transpose(1, 0, 2)
        )
        in_maps.append(
            {
                "x_in": np.ascontiguousarray(x_t).astype(bf),
                "wqk": np.ascontiguousarray(wqk_t).astype(bf),
                "wv": np.ascontiguousarray(wv_t).astype(bf),
                "wp": np.ascontiguousarray(wp_t).astype(bf),
            }
        )
    return in_maps


def run(in_maps, **kwargs):
    nc = _get_nc()
    return run_bass_kernel_spmd(nc, in_maps, core_ids=list(range(N_CORES)), **kwargs)


def combine(results):
    out = np.zeros((B, T, C), np.float64)
    for c in range(N_CORES):
        out[c // 4] += results[c]["out"].astype(np.float64)
    return out.astype(np.float32)


def kernel(x, w_attn, w_proj):
    res = run(make_in_maps(x, w_attn, w_proj))
    return combine(res.results)
